# revision 28
# baseline (speedup 1.0000x reference)
"""Trainium2 Bass kernel for per-position grouped-query attention.

Reference computation (B=4, S=4096, HID=2048, H=16, G=4, D=128, KV=512):
    q = x @ Wq + bq ; k = x @ Wk + bk ; v = x @ Wv + bv
    scores[t,h,g] = <q[t,h,:], k[t,g,:]> / sqrt(D)     (same-position only)
    probs = softmax_g(scores)
    o[t,h,:] = sum_g probs[t,h,g] * v[t,g,:]
    y = o @ Wo + bo
Since softmax is over G=4 groups at the same position, the work is dominated
by the QKV / Wo GEMMs (~43 GFLOP/core in bf16 -> ~550us at 78.6 TF/s).

Strategy: data-parallel over the 16384 flattened tokens -> 2048 tokens/core
on 8 cores, weights replicated, no collectives.  x arrives host-pre-transposed
(feature-major) so no PE transposes are needed on the input side.  Per
128-token tile, fused pipeline: QKV GEMMs (PE, PSUM-rotating) -> grouped
softmax+combine on DVE in bf16 (2x mode) with PSUM->SBUF copies on the scalar
engine -> PE transposes of the 16 o-blocks -> Wo GEMM -> bf16 y out.
Emission is software-pipelined: tile t+1's QKV matmuls are emitted before
tile t's transposes/Wo so the PE never waits on the vector engines.
"""

import sys

import numpy as np

sys.path.insert(0, "/opt/trn_rl_repo")

import ml_dtypes  # noqa: E402
from contextlib import ExitStack  # noqa: E402

import concourse.bass as bass  # noqa: E402
import concourse.bacc as bacc  # noqa: E402
import concourse.mybir as mybir  # noqa: E402
import concourse.tile as tile  # noqa: E402
from concourse.bass import ds  # noqa: E402
from concourse.bass_utils import run_bass_kernel_spmd  # noqa: E402
from concourse.masks import make_identity  # noqa: E402

B, S, HID = 4, 4096, 2048
H, G = 16, 4
D = HID // H          # 128
KV = HID * G // H     # 512
NCORES = 8
NTOK = B * S          # 16384
TPC = NTOK // NCORES  # 2048 tokens per core
P = 128
NTT = TPC // P        # 16 token tiles per core
NI = HID // P         # 16 input-feature blocks
NS = 6                # qkv output sections of 512 (4 q + 1 k + 1 v)
SCALE = 1.0 / float(np.sqrt(D))

BF16 = mybir.dt.bfloat16
F32 = mybir.dt.float32
_cache = {}


def _build(has_bias: bool, repeat: int = 1, strip_attn: bool = False) -> bass.Bass:
    mult = mybir.AluOpType.mult
    add = mybir.AluOpType.add

    nc = bacc.Bacc("TRN2")
    xfm = nc.dram_tensor("xfm", [HID, TPC], BF16, kind="ExternalInput")
    wq = nc.dram_tensor("wq", [HID, HID], BF16, kind="ExternalInput")
    wk = nc.dram_tensor("wk", [HID, KV], BF16, kind="ExternalInput")
    wv = nc.dram_tensor("wv", [HID, KV], BF16, kind="ExternalInput")
    wo = nc.dram_tensor("wo", [HID, HID], BF16, kind="ExternalInput")
    if has_bias:
        bqkv = nc.dram_tensor("bqkv", [1, HID + 2 * KV], F32, kind="ExternalInput")
        bo = nc.dram_tensor("bo", [1, HID], F32, kind="ExternalInput")
    y = nc.dram_tensor("y", [TPC, HID], BF16, kind="ExternalOutput")

    with tile.TileContext(nc) as tc, ExitStack() as ctx:
        const_pool = ctx.enter_context(tc.tile_pool(name="const", bufs=1))
        ident = const_pool.tile([P, P], BF16)
        make_identity(nc, ident[:])

        if has_bias:
            bias_qkv = const_pool.tile([P, HID + 2 * KV], F32)
            nc.sync.dma_start(bias_qkv[:], bqkv[0:1, :].broadcast_to((P, HID + 2 * KV)))
            bias_o = const_pool.tile([P, HID], F32)
            nc.sync.dma_start(bias_o[:], bo[0:1, :].broadcast_to((P, HID)))

        # weight residency (SBUF): 160KB/partition total
        w_pool = ctx.enter_context(tc.tile_pool(name="wgt", bufs=1))

        # PSUM pools: mmA (QKV) 3 banks, mmB (Wo) 4 banks, pt (transposes)
        mmA_pool = ctx.enter_context(tc.tile_pool(name="mmA", bufs=2, space="PSUM"))
        mmB_pool = ctx.enter_context(tc.tile_pool(name="mmB", bufs=2, space="PSUM"))
        pt_pool = ctx.enter_context(tc.tile_pool(name="pt", bufs=2, space="PSUM"))

        for rep in range(repeat):
            with tc.tile_pool(name=f"xf{rep}", bufs=2) as xf_pool, \
                 tc.tile_pool(name=f"qk{rep}", bufs=2) as qkv_pool, \
                 tc.tile_pool(name=f"at{rep}", bufs=1) as attn_pool, \
                 tc.tile_pool(name=f"ob{rep}", bufs=2) as obf_pool, \
                 tc.tile_pool(name=f"ot{rep}", bufs=2) as oT_pool, \
                 tc.tile_pool(name=f"yb{rep}", bufs=1) as ybf_pool:

                xf_tiles = {}
                qkv_tiles = {}

                def load_x(t):
                    xt = xf_pool.tile([P, NI, P], BF16)
                    nc.sync.dma_start(
                        xt[:],
                        xfm.rearrange("(i p) t -> p i t", p=P)[:, :, ds(t * P, P)],
                    )
                    xf_tiles[t] = xt

                psA_tiles = {}

                def qkv_mms(t):
                    xt = xf_tiles.pop(t)
                    qkv = qkv_pool.tile([P, NS * 512], BF16)
                    psA = []
                    for s in range(NS):
                        ps = mmA_pool.tile([P, 512], F32)
                        psA.append(ps)
                        for i in range(NI):
                            if s < 4:
                                rhs = wq_sb[i][:, s * 512:(s + 1) * 512]
                            elif s == 4:
                                rhs = wk_sb[i][:]
                            else:
                                rhs = wv_sb[i][:]
                            nc.tensor.matmul(
                                ps[:], xt[:, i, :], rhs,
                                start=(i == 0), stop=(i == NI - 1),
                            )
                    qkv_tiles[t] = qkv
                    psA_tiles[t] = psA

                def qkv_copies(t, lo, hi):
                    # PSUM->SBUF (+bf16 cast) on ACT, split into two chunks
                    # around exp(t-1) to avoid ACT head-of-line blocking
                    qkv = qkv_tiles[t]
                    psA = psA_tiles[t]
                    for s in range(lo, hi):
                        if has_bias:
                            nc.vector.tensor_add(
                                qkv[:, s * 512:(s + 1) * 512], psA[s][:],
                                bias_qkv[:, s * 512:(s + 1) * 512],
                            )
                        else:
                            nc.scalar.activation(
                                qkv[:, s * 512:(s + 1) * 512], psA[s][:],
                                mybir.ActivationFunctionType.Copy,
                            )
                    if hi == NS:
                        del psA_tiles[t]

                def attn_scores(t, tagsfx=""):
                    qkv = qkv_tiles[t]
                    # scores[t,h,g] = <q_h, k_g>*SCALE on DVE (bf16)
                    sc = attn_pool.tile([P, H * G], F32, tag=f"sc{tagsfx}")
                    junk = attn_pool.tile([P, D], BF16, tag="junk")
                    for h in range(H):
                        for g in range(G):
                            nc.vector.scalar_tensor_tensor(
                                junk[:],
                                qkv[:, h * D:(h + 1) * D],
                                SCALE,
                                qkv[:, HID + g * D:HID + (g + 1) * D],
                                op0=mult, op1=mult,
                                accum_out=sc[:, ds(h * G + g, 1)],
                            )
                    ex = attn_pool.tile([P, H * G], F32, tag=f"ex{tagsfx}")
                    nc.scalar.activation(ex[:], sc[:],
                                         mybir.ActivationFunctionType.Exp)
                    dn = attn_pool.tile([P, H], F32, tag=f"dn{tagsfx}")
                    nc.vector.reduce_sum(
                        dn[:], ex[:].rearrange("p (h g) -> p h g", g=G),
                        axis=mybir.AxisListType.X,
                    )
                    rc = attn_pool.tile([P, H], F32, tag=f"rc{tagsfx}")
                    nc.vector.reciprocal(rc[:], dn[:])
                    return ex, rc

                def attn_outproj(t, ex, rc):
                    qkv = qkv_tiles.pop(t)
                    # dv_g = v_g - v_0 (g=1..3); o_h = v0 + rc_h*sum_g e_hg dv_g
                    VOFF = HID + KV
                    if not strip_attn:
                        dv = attn_pool.tile([P, 3 * D], BF16, tag="dv")
                        for g in range(1, G):
                            nc.vector.tensor_sub(
                                dv[:, (g - 1) * D:g * D],
                                qkv[:, VOFF + g * D:VOFF + (g + 1) * D],
                                qkv[:, VOFF:VOFF + D],
                            )
                    obf = qkv[:, 0:HID] if strip_attn else obf_pool.tile([P, HID], BF16)
                    oT = oT_pool.tile([P, HID], BF16)
                    ta = attn_pool.tile([P, D], BF16, tag="ta")
                    tb = attn_pool.tile([P, D], BF16, tag="tb")

                    def combine_group(j):
                        if strip_attn:
                            return
                        for h in range(4 * j, 4 * j + 4):
                            nc.vector.tensor_scalar_mul(
                                ta[:], dv[:, 0:D], ex[:, ds(h * G + 1, 1)])
                            nc.vector.scalar_tensor_tensor(
                                tb[:], dv[:, D:2 * D], ex[:, ds(h * G + 2, 1)],
                                ta[:], op0=mult, op1=add)
                            nc.vector.scalar_tensor_tensor(
                                ta[:], dv[:, 2 * D:3 * D], ex[:, ds(h * G + 3, 1)],
                                tb[:], op0=mult, op1=add)
                            nc.vector.scalar_tensor_tensor(
                                obf[:, h * D:(h + 1) * D], ta[:], rc[:, ds(h, 1)],
                                qkv[:, VOFF:VOFF + D], op0=mult, op1=add)

                    def transp_group(j):
                        # 4 transposes (~1.1us) per burst, interleaved with Wo
                        # matmul groups so PE-busy gaps stay under the HAM window
                        pt = pt_pool.tile([P, 512], BF16)
                        for k in range(4):
                            blk = 4 * j + k
                            nc.tensor.transpose(
                                pt[:, k * P:(k + 1) * P],
                                obf[:, blk * P:(blk + 1) * P],
                                ident[:],
                            )
                        nc.scalar.activation(
                            oT[:, j * 512:(j + 1) * 512], pt[:],
                            mybir.ActivationFunctionType.Copy,
                        )

                    # per-j-group pipeline: combine 4 heads (DVE) -> transpose
                    # group (PE) -> Wo matmuls.  Wo runs as two 2-section
                    # passes (2 PSUM banks) so pt keeps 2 buffers; pass 0
                    # interleaves with the transpose groups, pass 1 is a pure
                    # PE run with no vector-engine dependencies (short tail).
                    ybf = ybf_pool.tile([P, HID], BF16)
                    combine_group(0)
                    for half in range(2):
                        psB = []
                        for sy in range(2):
                            psb_t = mmB_pool.tile([P, 512], F32, tag=f"y{sy}")
                            psB.append(psb_t)
                        for j in range(4):
                            if half == 0:
                                if j + 1 < 4:
                                    combine_group(j + 1)
                                transp_group(j)
                            for o in range(4 * j, 4 * j + 4):
                                for sy in range(2):
                                    sec = 2 * half + sy
                                    nc.tensor.matmul(
                                        psB[sy][:], oT[:, o * P:(o + 1) * P],
                                        wo_sb[o][:, sec * 512:(sec + 1) * 512],
                                        start=(o == 0), stop=(o == NI - 1),
                                    )
                        for sy in range(2):
                            sec = 2 * half + sy
                            if has_bias:
                                nc.vector.tensor_add(
                                    ybf[:, sec * 512:(sec + 1) * 512], psB[sy][:],
                                    bias_o[:, sec * 512:(sec + 1) * 512],
                                )
                            else:
                                nc.scalar.activation(
                                    ybf[:, sec * 512:(sec + 1) * 512], psB[sy][:],
                                    mybir.ActivationFunctionType.Copy,
                                )
                    nc.sync.dma_start(y[t * P:(t + 1) * P, :], ybf[:])

                # DMA order: x tiles for the first two tiles FIRST so the PE
                # can start as soon as wq blocks drip in (QKV matmuls pace on
                # per-block weight arrivals), then weights in use order.
                load_x(0)
                load_x(1)
                # weight DMAs round-robin over two DGE queues (SP + Pool) so
                # delivery parallelizes across DMA rings during startup
                wq_sb, wk_sb, wv_sb, wo_sb = [], [], [], []
                for i in range(NI):
                    t_ = w_pool.tile([P, HID], BF16, tag=f"wq{i}")
                    (nc.sync if i % 2 == 0 else nc.gpsimd).dma_start(
                        t_[:], wq[i * P:(i + 1) * P, :])
                    wq_sb.append(t_)
                for i in range(NI):
                    t_ = w_pool.tile([P, KV], BF16, tag=f"wk{i}")
                    (nc.sync if i % 2 == 0 else nc.gpsimd).dma_start(
                        t_[:], wk[i * P:(i + 1) * P, :])
                    wk_sb.append(t_)
                    t_ = w_pool.tile([P, KV], BF16, tag=f"wv{i}")
                    (nc.sync if i % 2 == 1 else nc.gpsimd).dma_start(
                        t_[:], wv[i * P:(i + 1) * P, :])
                    wv_sb.append(t_)
                for i in range(NI):
                    t_ = w_pool.tile([P, HID], BF16, tag=f"wo{i}")
                    (nc.sync if i % 2 == 0 else nc.gpsimd).dma_start(
                        t_[:], wo[i * P:(i + 1) * P, :])
                    wo_sb.append(t_)

                # software-pipelined emission: PE runs QKV(t+1) while the
                # vector engines run tile t's attention; transposes/Wo for
                # tile t interleave per j-group as combine results land.
                # ACT order per tile: copies s0-2(t+1), exp(t), copies s3-5
                # (t+1), oT copies(t) -- no cross-tile head-of-line blocking.
                qkv_mms(0)
                qkv_copies(0, 0, NS)
                last = None
                for t in range(NTT):
                    if t + 1 < NTT:
                        qkv_mms(t + 1)
                        qkv_copies(t + 1, 0, 3)
                    if t + 2 < NTT:
                        load_x(t + 2)
                    if strip_attn:
                        # perf-probe variant: same PE work, no DVE attention
                        if t + 1 < NTT:
                            qkv_copies(t + 1, 3, NS)
                        attn_outproj(t, None, None)
                        continue
                    if last is None:
                        ex, rc = attn_scores(t)
                    else:
                        ex, rc = last
                    if t + 1 < NTT:
                        qkv_copies(t + 1, 3, NS)
                    if t == NTT - 2:
                        # prefetch the last tile's scores on DVE/ACT before
                        # this tile's combine, shrinking the pipeline tail
                        last = attn_scores(t + 1, tagsfx="L")
                    attn_outproj(t, ex, rc)

    nc.compile()
    return nc


def prepare_in_maps(hidden_states, Wq, bq, Wk, bk, Wv, bv, Wo, bo):
    bf = ml_dtypes.bfloat16
    has_bias = bool(np.any(bq) or np.any(bk) or np.any(bv) or np.any(bo))
    x_flat = np.asarray(hidden_states, dtype=np.float32).reshape(NTOK, HID)
    wq_b = np.asarray(Wq, dtype=np.float32).astype(bf)
    wk_b = np.asarray(Wk, dtype=np.float32).astype(bf)
    wv_b = np.asarray(Wv, dtype=np.float32).astype(bf)
    wo_b = np.asarray(Wo, dtype=np.float32).astype(bf)

    in_maps = []
    for c in range(NCORES):
        xc = x_flat[c * TPC:(c + 1) * TPC]            # [TPC, HID] f32
        x_fm = np.ascontiguousarray(xc.T).astype(bf)  # [HID, TPC] bf16
        m = {"xfm": x_fm, "wq": wq_b, "wk": wk_b, "wv": wv_b, "wo": wo_b}
        if has_bias:
            m["bqkv"] = np.concatenate([
                np.asarray(bq, np.float32), np.asarray(bk, np.float32),
                np.asarray(bv, np.float32)]).reshape(1, HID + 2 * KV)
            m["bo"] = np.asarray(bo, np.float32).reshape(1, HID)
        in_maps.append(m)
    return has_bias, in_maps


def postprocess(results):
    out = np.concatenate([np.asarray(r["y"]) for r in results], axis=0)
    return out.astype(np.float32).reshape(B, S, HID)


def kernel(hidden_states, Wq, bq, Wk, bk, Wv, bv, Wo, bo, _profile=None):
    has_bias, in_maps = prepare_in_maps(
        hidden_states, Wq, bq, Wk, bk, Wv, bv, Wo, bo)
    if has_bias not in _cache:
        _cache[has_bias] = _build(has_bias)
    nc = _cache[has_bias]

    kwargs = dict(_profile) if _profile else {}
    kwargs.pop("result", None)
    res = run_bass_kernel_spmd(nc, in_maps, list(range(NCORES)), **kwargs)
    if _profile is not None:
        _profile["result"] = res
    return postprocess(res.results)
